# revision 14
# baseline (speedup 1.0000x reference)
"""Trainium2 Bass kernel for nn_Bilinear_31379031065270.

out[b,i,j,:] = sbp[b, Yi[b,i,j], Xi[b,i,j], :] where
  sbp = zero-padded 4-corner average of img = x[...,:3],
  Xi = floor((j + dx) % 224), Yi = floor((i + dy) % 224).

Strategy (per NeuronCore, 16 images each, 8 cores data-parallel):
 - compute sbp on-chip (DVE), write it to a DRAM buffer with one 256-byte
   slot per pixel (236 rows: 224 + 6 wrap rows on each side),
 - compute flat gather indices on-chip exactly as the reference does,
 - per-pixel gather via GPSIMD dma_gather with 12-byte elements at a
   256-byte index stride (the Q7 ucode supports this; bass's 256-byte
   elem_size assert is transpose-only, so we emit the instruction directly).
"""
import sys

sys.path.insert(0, "/opt/trn_rl_repo")

import numpy as np

import concourse.bacc as bacc
import concourse.bass as bass
import concourse.mybir as mybir
from concourse.bass import AP
from concourse.bass_utils import run_bass_kernel_spmd
from concourse.library_config import mlp
from concourse._compat import exact_div, round_up_to_multiple
from concourse.ap_utils import ap_is_contiguous

H = W = 224
PIX = H * W            # 50176 pixels per image
BPC = 16               # images per core
NCORES = 8
Q4 = 4                 # gather calls (quarters) per image
QPIX = PIX // Q4       # 12544
NIDX = QPIX + 128      # 12672 incl pad chunk
ICOLS = NIDX // 16     # 792
DCOLS = QPIX // 16     # 784 data cols
TROW = 236             # padded buffer rows: 6 wrap + 224 + 6 wrap
SLOTS = TROW * W       # 52864
ROWF = W * 64          # f32 elements per padded row (224 slots x 64)
XROW = W * 5           # 1120 f32 per image row of x
OPP = 98               # output pixels per partition per quarter


def dma_gather_raw(gpsimd, out_ap, in_ap, idxs_ap, num_idxs, elem_size, elem_step,
                   single_packet=False):
    """dma_gather with arbitrary elem_size (stride must be 256B-multiple)."""
    self = gpsimd
    assert idxs_ap.dtype == mybir.dt.int16
    assert ap_is_contiguous(out_ap.ap[1:])
    assert ap_is_contiguous(idxs_ap.ap[1:])
    assert in_ap.ap[-1][1] == out_ap.ap[-1][1] == elem_size
    assert out_ap.ap[0][1] * out_ap.ap[1][1] == round_up_to_multiple(num_idxs, 128)
    assert in_ap.ap[0][0] == elem_step
    stride_bytes = elem_step * mybir.dt.size(in_ap.dtype)
    stride_bytes_256 = exact_div(stride_bytes, 256)
    _in_ap = self.lower_ap_dma(in_ap, for_custom_bir_dma=True)
    _idxs_ap = self.lower_ap(idxs_ap)
    _out_ap = self.lower_ap(out_ap)
    return self.add_instruction(
        mybir.InstDMAGatherAnt(
            name=self.bass.get_next_instruction_name(),
            ins=[*_in_ap, _idxs_ap, self.lower_val_access(self.to_reg(num_idxs))],
            outs=[_out_ap],
            transpose=False,
            num_idxs=num_idxs,
            elem_size=elem_size,
            stride_bytes_256=stride_bytes_256,
            gen_mode=0,
            single_packet=single_packet,
            queue_num=0,
            sbuf_tokens_per_rank=0,
            sbuf_free_dim_per_rank=0,
            sbuf_free_dim_pad_per_rank=0,
            sbuf_byte_offset=0,
        )
    )


def build_nc():
    nc = bacc.Bacc("TRN2", debug=False, detect_race_conditions=False)
    dt = mybir.dt
    ALU = mybir.AluOpType
    x_t = nc.dram_tensor("x", [BPC * PIX + W, 5], dt.float32, kind="ExternalInput")
    jx_t = nc.dram_tensor("jx", [128, ICOLS], dt.float32, kind="ExternalInput")
    iy_t = nc.dram_tensor("iy", [128, ICOLS], dt.float32, kind="ExternalInput")
    out_t = nc.dram_tensor("out", [BPC * PIX, 3], dt.float32, kind="ExternalOutput")
    pad_t = nc.dram_tensor("padbuf", [SLOTS, 64], dt.float32, kind="Internal")

    from contextlib import ExitStack
    es = ExitStack()
    with es:
        block = es.enter_context(nc.Block())
        def sb(name, shape, dtp):
            return es.enter_context(nc.sbuf_tensor(name, shape, dtp))
        pt = sb("pt", [128, W * 64], dt.float32)
        xt1 = sb("xt1", [128, XROW], dt.float32)
        xt2 = sb("xt2", [128, XROW], dt.float32)
        cp = sb("cp", [128, 222 * 3], dt.float32)
        cpd = sb("cpd", [128, 222 * 3], dt.float32)
        xq = sb("xq", [128, 3960], dt.float32)
        jxs = sb("jxs", [128, ICOLS], dt.float32)
        iys = sb("iys", [128, ICOLS], dt.float32)
        t1 = sb("t1", [128, ICOLS], dt.float32)
        t2 = sb("t2", [128, ICOLS], dt.float32)
        t3 = sb("t3", [128, ICOLS], dt.float32)
        t4 = sb("t4", [128, ICOLS], dt.float32)
        t5 = sb("t5", [128, ICOLS], dt.float32)
        xip = sb("xip", [128, ICOLS], dt.float32)
        yip = sb("yip", [128, ICOLS], dt.float32)
        idx16 = sb("idx16", [128, ICOLS], dt.int16)
        idx2 = sb("idx2", [128, Q4 * ICOLS], dt.int16)
        outt = sb("outt", [128, Q4 * (OPP + 1), 3], dt.float32)
        s_l = es.enter_context(nc.semaphore("s_l"))
        s_v = es.enter_context(nc.semaphore("s_v"))
        s_w = es.enter_context(nc.semaphore("s_w"))
        s_g = es.enter_context(nc.semaphore("s_g"))
        s_o = es.enter_context(nc.semaphore("s_o"))
        s_c = es.enter_context(nc.semaphore("s_c"))
        def strided(tile_ap, off, dims):
            return AP(tile_ap.tensor, tile_ap.offset + off, [list(tile_ap.ap[0])] + dims)

        @block.vector
        def _(v):
            # one-time zeroing: pt partition 127 stays all-zero forever (border
            # rows); col-border slots 0/223 on all partitions
            v.memset(pt[0:32, :], 0.0)
            v.memset(pt[32:64, :], 0.0)
            v.memset(pt[64:96, :], 0.0)
            v.memset(pt[96:128, :], 0.0)
            v.memset(idx2[0:32, :], 0)
            v.memset(idx2[32:64, :], 0)
            v.memset(idx2[64:96, :], 0)
            v.memset(idx2[96:128, :], 0)

            for b in range(BPC):
                v.wait_ge(s_l, 32 + 160 * (b + 1))
                # ---- gather indices (phi layout [128, ICOLS]) ----
                dxv = strided(xq[:], 3, [[5, 99], [495, 8]])
                dyv = strided(xq[:], 4, [[5, 99], [495, 8]])
                for (posp, dv, resp) in ((jxs, dxv, xip), (iys, dyv, yip)):
                    v.tensor_tensor(strided(t1[:], 0, [[8, 99], [1, 8]]),
                                    strided(posp[:], 0, [[8, 99], [1, 8]]), dv,
                                    ALU.add)
                    v.tensor_scalar(t2[:, :], t1[:, :], 0.0, None, ALU.is_lt)
                    v.tensor_scalar(t3[:, :], t1[:, :], 224.0, None, ALU.is_ge)
                    v.tensor_tensor(t2[:, :], t2[:, :], t3[:, :], ALU.subtract)
                    v.scalar_tensor_tensor(t4[:, :], t2[:, :], 224.0, t1[:, :],
                                           ALU.mult, ALU.add)
                    i32v = t5[:, :].bitcast(dt.int32)
                    v.tensor_copy(out=i32v, in_=t4[:, :])
                    v.tensor_copy(out=t1[:, :], in_=i32v)
                    v.tensor_tensor(t2[:, :], t1[:, :], t4[:, :], ALU.is_gt)
                    v.tensor_tensor(t1[:, :], t1[:, :], t2[:, :], ALU.subtract)
                    v.tensor_scalar(resp[:, :], t1[:, :], 223.0, None, ALU.min)
                # t = Yi + 6 - 56Q (per 32-partition quarter), wrapped to [0,224)
                for q in range(Q4):
                    sl = slice(32 * q, 32 * (q + 1))
                    v.tensor_scalar(yip[sl, :], yip[sl, :], float(6 - 56 * q), None,
                                    ALU.add)
                v.tensor_scalar(t2[:, :], yip[:, :], 0.0, None, ALU.is_lt)
                v.tensor_scalar(t3[:, :], yip[:, :], 224.0, None, ALU.is_ge)
                v.tensor_tensor(t2[:, :], t2[:, :], t3[:, :], ALU.subtract)
                v.scalar_tensor_tensor(t4[:, :], t2[:, :], 224.0, yip[:, :],
                                       ALU.mult, ALU.add)
                v.scalar_tensor_tensor(t1[:, :], t4[:, :], 224.0, xip[:, :],
                                       ALU.mult, ALU.add)
                v.tensor_copy(out=idx16[:, :], in_=t1[:, :]).then_inc(s_v, 1)
                # ---- sbp part 1 (image rows 1..126 from xt1) ----
                a0 = strided(xt1[:], 0, [[5, 222], [1, 3]])
                a2 = strided(xt1[:], 10, [[5, 222], [1, 3]])
                v.tensor_tensor(cp[:, :], a0, a2, ALU.add)
                v.tensor_scalar(cp[:, :], cp[:, :], 0.25, None,
                                ALU.mult).then_inc(s_v, 1)
                v.wait_ge(s_c, 32 * b + 16)
                v.wait_ge(s_w, 192 * b)
                v.tensor_tensor(
                    strided(pt[0:126], 64, [[64, 222], [1, 3]]),
                    cp[0:126, :], cpd[0:126, :], ALU.add,
                ).then_inc(s_v, 1)
                # ---- sbp part 2 (image rows 97..222 from xt2) ----
                v.wait_ge(s_w, 192 * b + 128)
                b0 = strided(xt2[:], 0, [[5, 222], [1, 3]])
                b2 = strided(xt2[:], 10, [[5, 222], [1, 3]])
                v.tensor_tensor(cp[:, :], b0, b2, ALU.add)
                v.tensor_scalar(cp[:, :], cp[:, :], 0.25, None,
                                ALU.mult).then_inc(s_v, 1)
                v.wait_ge(s_c, 32 * b + 32)
                v.tensor_tensor(
                    strided(pt[0:126], 64, [[64, 222], [1, 3]]),
                    cp[0:126, :], cpd[0:126, :], ALU.add,
                ).then_inc(s_v, 1)

        @block.gpsimd
        def _(g):
            g.load_library(mlp)
            es.enter_context(nc.allow_non_contiguous_dma(
                reason="strided per-quarter x loads"))
            g.dma_start(jxs[:, :], jx_t[:, :]).then_inc(s_l, 16)
            g.dma_start(iys[:, :], iy_t[:, :]).then_inc(s_l, 16)
            for b in range(BPC):
                xoff = b * PIX * 5
                g.wait_ge(s_v, 5 * b)
                # loads: 2 x-row tiles + 8 per-quarter chunks
                g.dma_start(xt1[:, :],
                            AP(x_t, xoff, [[XROW, 128], [1, XROW]])).then_inc(s_l, 16)
                g.dma_start(xt2[:, :],
                            AP(x_t, xoff + 96 * XROW,
                               [[XROW, 128], [1, XROW]])).then_inc(s_l, 16)
                for q in range(Q4):
                    for r in range(2):
                        psl = xq[32 * q + 16 * r: 32 * q + 16 * r + 16]
                        srcap = AP(x_t, xoff + q * QPIX * 5,
                                   [[98 * 5, 16], [16 * 98 * 5, 8], [1, 495]])
                        g.dma_start(psl[:, :], srcap).then_inc(s_l, 16)
                g.wait_ge(s_v, 5 * b + 1)
                # fold idx16 -> idx2 (per-quarter lists on partitions 0..31)
                for q in range(Q4):
                    g.dma_start(idx2[0:32, q * ICOLS: q * ICOLS + DCOLS],
                                idx16[32 * q: 32 * q + 32, 0:DCOLS]).then_inc(s_w, 16)
                g.wait_ge(s_v, 5 * b + 2)
                g.dma_start(cpd[0:126, :], cp[2:128, :]).then_inc(s_c, 16)
                g.wait_ge(s_v, 5 * b + 3)
                # pad writes part 1: row0 zeros -> t=6; rows 1..126 -> t=7..132;
                # wrap-high: row 0 -> t=230, rows 1..5 -> t=231..235
                g.dma_start(AP(pad_t, 6 * ROWF, [[ROWF, 1], [1, ROWF]]),
                            pt[127:128, :]).then_inc(s_w, 16)
                g.dma_start(AP(pad_t, 7 * ROWF, [[ROWF, 126], [1, ROWF]]),
                            pt[0:126, :]).then_inc(s_w, 16)
                g.dma_start(AP(pad_t, 230 * ROWF, [[ROWF, 1], [1, ROWF]]),
                            pt[127:128, :]).then_inc(s_w, 16)
                g.dma_start(AP(pad_t, 231 * ROWF, [[ROWF, 5], [1, ROWF]]),
                            pt[0:5, :]).then_inc(s_w, 16)
                g.wait_ge(s_v, 5 * b + 4)
                g.dma_start(cpd[0:126, :], cp[2:128, :]).then_inc(s_c, 16)
                g.wait_ge(s_v, 5 * b + 5)
                # pad writes part 2: rows 97..222 -> t=103..228; row 223 zeros
                # -> t=229; wrap-low: rows 218..222 -> t=0..4, row 223 -> t=5
                g.dma_start(AP(pad_t, 103 * ROWF, [[ROWF, 126], [1, ROWF]]),
                            pt[0:126, :]).then_inc(s_w, 16)
                g.dma_start(AP(pad_t, 229 * ROWF, [[ROWF, 1], [1, ROWF]]),
                            pt[127:128, :]).then_inc(s_w, 16)
                g.dma_start(AP(pad_t, 0, [[ROWF, 5], [1, ROWF]]),
                            pt[121:126, :]).then_inc(s_w, 16)
                g.dma_start(AP(pad_t, 5 * ROWF, [[ROWF, 1], [1, ROWF]]),
                            pt[127:128, :]).then_inc(s_w, 16)
                g.wait_ge(s_w, 192 * (b + 1))
                # gathers (4 quarters)
                for q in range(Q4):
                    srcg = AP(pad_t, q * QPIX * 64, [[64, 15232], [1, 3]])
                    dma_gather_raw(
                        g, outt[:, q * (OPP + 1): (q + 1) * (OPP + 1), :], srcg,
                        idx2[:, q * ICOLS: (q + 1) * ICOLS], NIDX, 3, 64,
                    ).then_inc(s_g, 16)
                g.wait_ge(s_g, 64 * (b + 1))
                # outputs
                for q in range(Q4):
                    dstq = AP(out_t, (b * PIX + q * QPIX) * 3,
                              [[OPP * 3, 128], [1, OPP * 3]])
                    g.dma_start(dstq, outt[:, q * (OPP + 1): q * (OPP + 1) + OPP, :]
                                ).then_inc(s_o, 16)
            g.wait_ge(s_o, 64 * BPC)

    nc.compile()
    return nc


def host_constants():
    """jx/iy planes in the phi layout: tile (P, col) holds output pixel
    pi = Q*QPIX + (16*w + s)*98 + e  with Q=P//32, s=P%16, w=col%8, e=col//8."""
    P = np.arange(128)[:, None]
    col = np.arange(ICOLS)[None, :]
    Qq = P // 32
    s = P % 16
    w = col % 8
    e = col // 8
    pi = Qq * QPIX + (16 * w + s) * OPP + e
    pi = np.minimum(pi, PIX - 1)
    jx = (pi % W).astype(np.float32)
    iy = (pi // W).astype(np.float32)
    return jx, iy


_NC = None


def kernel(x):
    global _NC
    x = np.asarray(x, dtype=np.float32)
    B = x.shape[0]
    assert x.shape == (B, H, W, 5) and B == NCORES * BPC
    if _NC is None:
        _NC = build_nc()
    jx, iy = host_constants()
    pad = np.zeros((W, 5), dtype=np.float32)
    in_maps = []
    for c in range(NCORES):
        xc = x[c * BPC:(c + 1) * BPC].reshape(-1, 5)
        xc = np.concatenate([xc, pad], axis=0)
        in_maps.append({"x": xc, "jx": jx, "iy": iy})
    res = run_bass_kernel_spmd(_NC, in_maps, core_ids=list(range(NCORES)))
    outs = [res.results[c]["out"].reshape(BPC, H, W, 3) for c in range(NCORES)]
    return np.concatenate(outs, axis=0)


# revision 15
# speedup vs baseline: 1.1760x; 1.1760x over previous
"""Trainium2 Bass kernel for nn_Bilinear_31379031065270.

out[b,i,j,:] = sbp[b, Yi[b,i,j], Xi[b,i,j], :] where
  sbp = zero-padded 4-corner average of img = x[...,:3],
  Xi = floor((j + dx) % 224), Yi = floor((i + dy) % 224).

Strategy (per NeuronCore, 16 images each, 8 cores data-parallel):
 - compute sbp on-chip (DVE), write it to a DRAM buffer with one 256-byte
   slot per pixel (236 rows: 224 + 6 wrap rows on each side),
 - compute flat gather indices on-chip exactly as the reference does,
 - per-pixel gather via GPSIMD dma_gather with 12-byte elements at a
   256-byte index stride (the Q7 ucode supports this; bass's 256-byte
   elem_size assert is transpose-only, so we emit the instruction directly).
"""
import sys

sys.path.insert(0, "/opt/trn_rl_repo")

import numpy as np

import concourse.bacc as bacc
import concourse.bass as bass
import concourse.mybir as mybir
from concourse.bass import AP
from concourse.bass_utils import run_bass_kernel_spmd
from concourse.library_config import mlp
from concourse._compat import exact_div, round_up_to_multiple
from concourse.ap_utils import ap_is_contiguous

H = W = 224
PIX = H * W            # 50176 pixels per image
BPC = 16               # images per core
NCORES = 8
Q4 = 4                 # gather calls (quarters) per image
QPIX = PIX // Q4       # 12544
NIDX = QPIX + 128      # 12672 incl pad chunk
ICOLS = NIDX // 16     # 792
DCOLS = QPIX // 16     # 784 data cols
TROW = 236             # padded buffer rows: 6 wrap + 224 + 6 wrap
SLOTS = TROW * W       # 52864
ROWF = W * 64          # f32 elements per padded row (224 slots x 64)
XROW = W * 5           # 1120 f32 per image row of x
OPP = 98               # output pixels per partition per quarter


def dma_gather_raw(gpsimd, out_ap, in_ap, idxs_ap, num_idxs, elem_size, elem_step,
                   single_packet=False):
    """dma_gather with arbitrary elem_size (stride must be 256B-multiple)."""
    self = gpsimd
    assert idxs_ap.dtype == mybir.dt.int16
    assert ap_is_contiguous(out_ap.ap[1:])
    assert ap_is_contiguous(idxs_ap.ap[1:])
    assert in_ap.ap[-1][1] == out_ap.ap[-1][1] == elem_size
    assert out_ap.ap[0][1] * out_ap.ap[1][1] == round_up_to_multiple(num_idxs, 128)
    assert in_ap.ap[0][0] == elem_step
    stride_bytes = elem_step * mybir.dt.size(in_ap.dtype)
    stride_bytes_256 = exact_div(stride_bytes, 256)
    _in_ap = self.lower_ap_dma(in_ap, for_custom_bir_dma=True)
    _idxs_ap = self.lower_ap(idxs_ap)
    _out_ap = self.lower_ap(out_ap)
    return self.add_instruction(
        mybir.InstDMAGatherAnt(
            name=self.bass.get_next_instruction_name(),
            ins=[*_in_ap, _idxs_ap, self.lower_val_access(self.to_reg(num_idxs))],
            outs=[_out_ap],
            transpose=False,
            num_idxs=num_idxs,
            elem_size=elem_size,
            stride_bytes_256=stride_bytes_256,
            gen_mode=0,
            single_packet=single_packet,
            queue_num=0,
            sbuf_tokens_per_rank=0,
            sbuf_free_dim_per_rank=0,
            sbuf_free_dim_pad_per_rank=0,
            sbuf_byte_offset=0,
        )
    )


def build_nc():
    nc = bacc.Bacc("TRN2", debug=False, detect_race_conditions=False)
    dt = mybir.dt
    ALU = mybir.AluOpType
    x_t = nc.dram_tensor("x", [BPC * PIX + W, 5], dt.float32, kind="ExternalInput")
    jx_t = nc.dram_tensor("jx", [128, ICOLS], dt.float32, kind="ExternalInput")
    iy_t = nc.dram_tensor("iy", [128, ICOLS], dt.float32, kind="ExternalInput")
    out_t = nc.dram_tensor("out", [BPC * PIX, 3], dt.float32, kind="ExternalOutput")
    pad_t0 = nc.dram_tensor("padbuf0", [SLOTS, 64], dt.float32, kind="Internal")
    pad_t1 = nc.dram_tensor("padbuf1", [SLOTS, 64], dt.float32, kind="Internal")

    from contextlib import ExitStack
    es = ExitStack()
    with es:
        block = es.enter_context(nc.Block())
        def sb(name, shape, dtp):
            return es.enter_context(nc.sbuf_tensor(name, shape, dtp))
        pt = sb("pt", [128, W * 64], dt.float32)
        xt1 = sb("xt1", [128, XROW], dt.float32)
        xt2 = sb("xt2", [128, XROW], dt.float32)
        cp = sb("cp", [128, 222 * 3], dt.float32)
        cpd = sb("cpd", [128, 222 * 3], dt.float32)
        xq = sb("xq", [128, 3960], dt.float32)
        jxs = sb("jxs", [128, ICOLS], dt.float32)
        iys = sb("iys", [128, ICOLS], dt.float32)
        t1 = sb("t1", [128, ICOLS], dt.float32)
        t2 = sb("t2", [128, ICOLS], dt.float32)
        t3 = sb("t3", [128, ICOLS], dt.float32)
        t4 = sb("t4", [128, ICOLS], dt.float32)
        t5 = sb("t5", [128, ICOLS], dt.float32)
        xip = sb("xip", [128, ICOLS], dt.float32)
        yip = sb("yip", [128, ICOLS], dt.float32)
        idx16 = sb("idx16", [128, ICOLS], dt.int16)
        idx2 = sb("idx2", [128, 2 * Q4 * ICOLS], dt.int16)
        outt = sb("outt", [128, 2 * Q4 * (OPP + 1), 3], dt.float32)
        s_l = es.enter_context(nc.semaphore("s_l"))
        s_v = es.enter_context(nc.semaphore("s_v"))
        s_w = es.enter_context(nc.semaphore("s_w"))
        s_g = es.enter_context(nc.semaphore("s_g"))
        s_o = es.enter_context(nc.semaphore("s_o"))
        s_c = es.enter_context(nc.semaphore("s_c"))
        def strided(tile_ap, off, dims):
            return AP(tile_ap.tensor, tile_ap.offset + off, [list(tile_ap.ap[0])] + dims)

        @block.vector
        def _(v):
            # one-time zeroing: pt partition 127 stays all-zero forever (border
            # rows); col-border slots 0/223 on all partitions
            v.memset(pt[0:32, :], 0.0)
            v.memset(pt[32:64, :], 0.0)
            v.memset(pt[64:96, :], 0.0)
            v.memset(pt[96:128, :], 0.0)
            v.memset(idx2[0:32, :], 0)
            v.memset(idx2[32:64, :], 0)
            v.memset(idx2[64:96, :], 0)
            v.memset(idx2[96:128, :], 0)
            # (both halves zeroed; folds only touch data cols)

            for b in range(BPC):
                v.wait_ge(s_l, 32 + 160 * (b + 1))
                # ---- gather indices (phi layout [128, ICOLS]) ----
                dxv = strided(xq[:], 3, [[5, 99], [495, 8]])
                dyv = strided(xq[:], 4, [[5, 99], [495, 8]])
                for (posp, dv, resp) in ((jxs, dxv, xip), (iys, dyv, yip)):
                    v.tensor_tensor(strided(t1[:], 0, [[8, 99], [1, 8]]),
                                    strided(posp[:], 0, [[8, 99], [1, 8]]), dv,
                                    ALU.add)
                    v.tensor_scalar(t2[:, :], t1[:, :], 0.0, None, ALU.is_lt)
                    v.tensor_scalar(t3[:, :], t1[:, :], 224.0, None, ALU.is_ge)
                    v.tensor_tensor(t2[:, :], t2[:, :], t3[:, :], ALU.subtract)
                    v.scalar_tensor_tensor(t4[:, :], t2[:, :], 224.0, t1[:, :],
                                           ALU.mult, ALU.add)
                    i32v = t5[:, :].bitcast(dt.int32)
                    v.tensor_copy(out=i32v, in_=t4[:, :])
                    v.tensor_copy(out=t1[:, :], in_=i32v)
                    v.tensor_tensor(t2[:, :], t1[:, :], t4[:, :], ALU.is_gt)
                    v.tensor_tensor(t1[:, :], t1[:, :], t2[:, :], ALU.subtract)
                    v.tensor_scalar(resp[:, :], t1[:, :], 223.0, None, ALU.min)
                # t = Yi + 6 - 56Q (per 32-partition quarter), wrapped to [0,224)
                for q in range(Q4):
                    sl = slice(32 * q, 32 * (q + 1))
                    v.tensor_scalar(yip[sl, :], yip[sl, :], float(6 - 56 * q), None,
                                    ALU.add)
                v.tensor_scalar(t2[:, :], yip[:, :], 0.0, None, ALU.is_lt)
                v.tensor_scalar(t3[:, :], yip[:, :], 224.0, None, ALU.is_ge)
                v.tensor_tensor(t2[:, :], t2[:, :], t3[:, :], ALU.subtract)
                v.scalar_tensor_tensor(t4[:, :], t2[:, :], 224.0, yip[:, :],
                                       ALU.mult, ALU.add)
                v.scalar_tensor_tensor(t1[:, :], t4[:, :], 224.0, xip[:, :],
                                       ALU.mult, ALU.add)
                v.tensor_copy(out=idx16[:, :], in_=t1[:, :]).then_inc(s_v, 1)
                # ---- sbp part 1 (image rows 1..126 from xt1) ----
                a0 = strided(xt1[:], 0, [[5, 222], [1, 3]])
                a2 = strided(xt1[:], 10, [[5, 222], [1, 3]])
                v.tensor_tensor(cp[:, :], a0, a2, ALU.add)
                v.tensor_scalar(cp[:, :], cp[:, :], 0.25, None,
                                ALU.mult).then_inc(s_v, 1)
                v.wait_ge(s_c, 32 * b + 16)
                v.wait_ge(s_w, 192 * b)
                v.tensor_tensor(
                    strided(pt[0:126], 64, [[64, 222], [1, 3]]),
                    cp[0:126, :], cpd[0:126, :], ALU.add,
                ).then_inc(s_v, 1)
                # ---- sbp part 2 (image rows 97..222 from xt2) ----
                v.wait_ge(s_w, 192 * b + 128)
                b0 = strided(xt2[:], 0, [[5, 222], [1, 3]])
                b2 = strided(xt2[:], 10, [[5, 222], [1, 3]])
                v.tensor_tensor(cp[:, :], b0, b2, ALU.add)
                v.tensor_scalar(cp[:, :], cp[:, :], 0.25, None,
                                ALU.mult).then_inc(s_v, 1)
                v.wait_ge(s_c, 32 * b + 32)
                v.tensor_tensor(
                    strided(pt[0:126], 64, [[64, 222], [1, 3]]),
                    cp[0:126, :], cpd[0:126, :], ALU.add,
                ).then_inc(s_v, 1)

        @block.gpsimd
        def _(g):
            g.load_library(mlp)
            for b in range(BPC):
                pad_b = pad_t0 if b % 2 == 0 else pad_t1
                ioff = (b % 2) * Q4 * ICOLS
                ooff = (b % 2) * Q4 * (OPP + 1)
                g.wait_ge(s_w, 192 * (b + 1))
                if b >= 2:
                    g.wait_ge(s_o, 64 * (b - 1))
                for q in range(Q4):
                    srcg = AP(pad_b, q * QPIX * 64, [[64, 15232], [1, 3]])
                    dma_gather_raw(
                        g, outt[:, ooff + q * (OPP + 1): ooff + (q + 1) * (OPP + 1), :],
                        srcg, idx2[:, ioff + q * ICOLS: ioff + (q + 1) * ICOLS],
                        NIDX, 3, 64,
                    ).then_inc(s_g, 16)
            g.wait_ge(s_g, 64 * BPC)

        @block.sync
        def _(g):
            es.enter_context(nc.allow_non_contiguous_dma(
                reason="strided per-quarter x loads"))
            g.dma_start(jxs[:, :], jx_t[:, :]).then_inc(s_l, 16)
            g.dma_start(iys[:, :], iy_t[:, :]).then_inc(s_l, 16)
            for b in range(BPC):
                pad_b = pad_t0 if b % 2 == 0 else pad_t1
                ioff = (b % 2) * Q4 * ICOLS
                xoff = b * PIX * 5
                g.wait_ge(s_v, 5 * b)
                g.dma_start(xt1[:, :],
                            AP(x_t, xoff, [[XROW, 128], [1, XROW]])).then_inc(s_l, 16)
                g.dma_start(xt2[:, :],
                            AP(x_t, xoff + 96 * XROW,
                               [[XROW, 128], [1, XROW]])).then_inc(s_l, 16)
                for q in range(Q4):
                    for r in range(2):
                        psl = xq[32 * q + 16 * r: 32 * q + 16 * r + 16]
                        srcap = AP(x_t, xoff + q * QPIX * 5,
                                   [[98 * 5, 16], [16 * 98 * 5, 8], [1, 495]])
                        g.dma_start(psl[:, :], srcap).then_inc(s_l, 16)
                g.wait_ge(s_v, 5 * b + 1)
                if b >= 2:
                    g.wait_ge(s_g, 64 * (b - 1))   # idx2 half free
                for q in range(Q4):
                    g.dma_start(idx2[0:32, ioff + q * ICOLS: ioff + q * ICOLS + DCOLS],
                                idx16[32 * q: 32 * q + 32, 0:DCOLS]).then_inc(s_w, 16)
                g.wait_ge(s_v, 5 * b + 2)
                g.dma_start(cpd[0:126, :], cp[2:128, :]).then_inc(s_c, 16)
                g.wait_ge(s_v, 5 * b + 3)
                if b >= 2:
                    g.wait_ge(s_g, 64 * (b - 1))   # pad_b free (gathers b-2 done)
                g.dma_start(AP(pad_b, 6 * ROWF, [[ROWF, 1], [1, ROWF]]),
                            pt[127:128, :]).then_inc(s_w, 16)
                g.dma_start(AP(pad_b, 7 * ROWF, [[ROWF, 126], [1, ROWF]]),
                            pt[0:126, :]).then_inc(s_w, 16)
                g.dma_start(AP(pad_b, 230 * ROWF, [[ROWF, 1], [1, ROWF]]),
                            pt[127:128, :]).then_inc(s_w, 16)
                g.dma_start(AP(pad_b, 231 * ROWF, [[ROWF, 5], [1, ROWF]]),
                            pt[0:5, :]).then_inc(s_w, 16)
                g.wait_ge(s_v, 5 * b + 4)
                g.dma_start(cpd[0:126, :], cp[2:128, :]).then_inc(s_c, 16)
                g.wait_ge(s_v, 5 * b + 5)
                g.dma_start(AP(pad_b, 103 * ROWF, [[ROWF, 126], [1, ROWF]]),
                            pt[0:126, :]).then_inc(s_w, 16)
                g.dma_start(AP(pad_b, 229 * ROWF, [[ROWF, 1], [1, ROWF]]),
                            pt[127:128, :]).then_inc(s_w, 16)
                g.dma_start(AP(pad_b, 0, [[ROWF, 5], [1, ROWF]]),
                            pt[121:126, :]).then_inc(s_w, 16)
                g.dma_start(AP(pad_b, 5 * ROWF, [[ROWF, 1], [1, ROWF]]),
                            pt[127:128, :]).then_inc(s_w, 16)
                # ship previous image's gathered output
                if b >= 1:
                    g.wait_ge(s_g, 64 * b)
                    po = ((b - 1) % 2) * Q4 * (OPP + 1)
                    for q in range(Q4):
                        dstq = AP(out_t, ((b - 1) * PIX + q * QPIX) * 3,
                                  [[OPP * 3, 128], [1, OPP * 3]])
                        g.dma_start(dstq,
                                    outt[:, po + q * (OPP + 1): po + q * (OPP + 1) + OPP, :]
                                    ).then_inc(s_o, 16)
            g.wait_ge(s_g, 64 * BPC)
            po = ((BPC - 1) % 2) * Q4 * (OPP + 1)
            for q in range(Q4):
                dstq = AP(out_t, ((BPC - 1) * PIX + q * QPIX) * 3,
                          [[OPP * 3, 128], [1, OPP * 3]])
                g.dma_start(dstq,
                            outt[:, po + q * (OPP + 1): po + q * (OPP + 1) + OPP, :]
                            ).then_inc(s_o, 16)
            g.wait_ge(s_o, 64 * BPC)

    nc.compile()
    return nc


def host_constants():
    """jx/iy planes in the phi layout: tile (P, col) holds output pixel
    pi = Q*QPIX + (16*w + s)*98 + e  with Q=P//32, s=P%16, w=col%8, e=col//8."""
    P = np.arange(128)[:, None]
    col = np.arange(ICOLS)[None, :]
    Qq = P // 32
    s = P % 16
    w = col % 8
    e = col // 8
    pi = Qq * QPIX + (16 * w + s) * OPP + e
    pi = np.minimum(pi, PIX - 1)
    jx = (pi % W).astype(np.float32)
    iy = (pi // W).astype(np.float32)
    return jx, iy


_NC = None


def kernel(x):
    global _NC
    x = np.asarray(x, dtype=np.float32)
    B = x.shape[0]
    assert x.shape == (B, H, W, 5) and B == NCORES * BPC
    if _NC is None:
        _NC = build_nc()
    jx, iy = host_constants()
    pad = np.zeros((W, 5), dtype=np.float32)
    in_maps = []
    for c in range(NCORES):
        xc = x[c * BPC:(c + 1) * BPC].reshape(-1, 5)
        xc = np.concatenate([xc, pad], axis=0)
        in_maps.append({"x": xc, "jx": jx, "iy": iy})
    res = run_bass_kernel_spmd(_NC, in_maps, core_ids=list(range(NCORES)))
    outs = [res.results[c]["out"].reshape(BPC, H, W, 3) for c in range(NCORES)]
    return np.concatenate(outs, axis=0)
